# revision 6
# baseline (speedup 1.0000x reference)
"""Trainium2 Bass kernel for nn_MultiHeadAttention_70549132804637.

Reference computation (note: NO softmax — values use raw scaled logits):
    qkv = x @ w_qkv.T                         (B,S,3E) split per head into q,k,v
    logits = (q @ k^T) * scale                (B,H,S,S)
    values = logits @ v                       (B,H,S,D)
    out    = concat_heads(values) @ w_o.T     (B,S,E)

Because there is no softmax the whole map is linear in x on the left:
    out[t] = sum_h x[t] Wq_h^T M_h Wo_h^T = x[t] @ W,   M_h = scale k_h^T v_h
so after computing the tiny per-head M_h matrices we can fold the q
projection and the output projection into one combined E x E matrix W
per batch.  This removes the entire q-projection GEMM (a quarter of the
baseline's FLOPs).

Sharding over 8 cores: core c handles batch b = c//2 and head-half
r = c%2 (8 of 16 heads).  Each core computes k,v (and hence M_h) for its
8 heads, then the two cores of a batch AllGather their M_h blocks
(128 KB) so both see all 16 heads.  Each core then builds its half of
the combined matrix W[:, r*512:(r+1)*512] and computes
out[:, r*512:(r+1)*512] = x_b @ W_half — the host simply concatenates
column halves (no partial-sum adds).

Per-core phases (T=2048 tokens, E=1024, 8 local heads, D=64):
  phase 1: kv[t, j] = xT-slices (stationary) x wkv (moving); per t-tile
           psM[pair] += k_pair^T @ v_pair (PSUM-resident, 16 t-tiles)
  cc     : scale + extract diag 64x64 blocks -> bf16 blockdiag send tile,
           pair AllGather via DRAM bounce, read back all 8 global pairs
  phase 2: AT[hd, e] = Mblk_g (stationary) x wq-rows_g (moving)
           W[e, oc]  = AT-slices (stationary) x woT_g (moving), acc over g
  phase 3: out[t, oc] = xT-slices (stationary) x W (moving), acc over e

All matmul operands bf16 (FWL fast weight loads); PSUM accumulates f32.
A short stream of dummy matmuls during the DMA lead-in keeps the PE's
HAM activity monitor busy so real matmuls start at the full clock.
"""

from contextlib import ExitStack

import numpy as np

import concourse.mybir as mybir
import concourse.tile as tile
from concourse import bacc
from concourse.bass import ts
from concourse.bass_utils import run_bass_kernel_spmd

F32 = mybir.dt.float32
BF16 = mybir.dt.bfloat16

B, S, E, H = 4, 2048, 1024, 16
D = E // H                 # 64 per-head dim
SCALE = 0.125              # 1/sqrt(D), exact power of two
NCORES = 8
HPC = H // 2               # heads per core = 8
P = 128
ET = E // P                # 8 e-tiles (contraction tiles for projections)
TC = 4                     # token chunks
TW = S // TC               # 512 tokens per chunk
TT = S // P                # 16 token tiles
NPAIR = HPC // 2           # 4 local head pairs per core
GPAIR = H // 2             # 8 global head pairs
JQ = HPC * D               # 512 kv columns (k) per core
OC = E // 2                # 512 output columns per core
PAIR_GROUPS = [[0, 1], [2, 3], [4, 5], [6, 7]]

_MODULE = None


def _emit(tc_, nc, xt, wkv, wq, wo, out):
    with ExitStack() as ctx:
        xp = ctx.enter_context(tc_.tile_pool(name="xp", bufs=ET * TC))
        wkvp = ctx.enter_context(tc_.tile_pool(name="wkvp", bufs=ET))
        wqp = ctx.enter_context(tc_.tile_pool(name="wqp", bufs=GPAIR))
        wop = ctx.enter_context(tc_.tile_pool(name="wop", bufs=GPAIR))
        kvp = ctx.enter_context(tc_.tile_pool(name="kvp", bufs=TT))
        msp = ctx.enter_context(tc_.tile_pool(name="msp", bufs=2))
        atp = ctx.enter_context(tc_.tile_pool(name="atp", bufs=GPAIR))
        wsp = ctx.enter_context(tc_.tile_pool(name="wsp", bufs=ET))
        otp = ctx.enter_context(tc_.tile_pool(name="otp", bufs=4))
        dram = ctx.enter_context(
            tc_.tile_pool(name="dram", bufs=2, space="DRAM"))

        # ---------------- DMA in ----------------
        # order: (wkv[ei], x[ei, chunk0]) pairs first so the kv matmuls can
        # start after ~400 KB; remaining x chunks stream behind compute.
        wkvsb = [wkvp.tile([P, 2 * JQ], BF16, name="wkvsb") for _ in range(ET)]
        xsb = [[xp.tile([P, TW], BF16, name="xsb") for _ in range(TC)]
               for _ in range(ET)]
        for ei in range(ET):
            nc.sync.dma_start(wkvsb[ei][:], wkv[ts(ei, P), :])
            nc.sync.dma_start(xsb[ei][0][:], xt[ts(ei, P), 0:TW])
        for c in range(1, TC):
            for ei in range(ET):
                nc.sync.dma_start(xsb[ei][c][:], xt[ts(ei, P), ts(c, TW)])
        wqsb = [wqp.tile([P, E], BF16, name="wqsb") for _ in range(GPAIR)]
        wosb = [wop.tile([P, OC], BF16, name="wosb") for _ in range(GPAIR)]
        for g in range(GPAIR):
            nc.sync.dma_start(wqsb[g][:], wq[ts(g, P), :])
        for g in range(GPAIR):
            nc.sync.dma_start(wosb[g][:], wo[ts(g, P), :])

        # PE warm-up: dummy matmuls during the DMA head keep the HAM
        # activity monitor busy so real matmuls start at full clock
        warm = ctx.enter_context(tc_.tile_pool(name="warm", bufs=1))
        wt = warm.tile([P, P], BF16, name="wt")
        nc.gpsimd.memset(wt[:], 0.0)
        with tc_.tile_pool(name="psW0", bufs=1, space="PSUM") as psW0:
            wps = psW0.tile([P, P], F32, name="wps")
            for _ in range(14):
                nc.tensor.matmul(wps[:], wt[:], wt[:], start=True, stop=True)

        # ---------------- phase 1: kv proj + M accumulation ----------------
        kvsb = [kvp.tile([P, 2 * JQ], BF16, name="kvsb") for _ in range(TT)]

        def macc(t, psMt):
            for p in range(NPAIR):
                nc.tensor.matmul(
                    psMt[p][:],
                    kvsb[t][:, ts(p, P)],
                    kvsb[t][:, JQ + p * P:JQ + (p + 1) * P],
                    start=(t == 0), stop=(t == TT - 1),
                    skip_group_check=True,
                )

        with tc_.tile_pool(name="psM", bufs=NPAIR, space="PSUM") as psM:
            psMt = [psM.tile([P, P], F32, name="psMt") for _ in range(NPAIR)]
            with tc_.tile_pool(name="psB", bufs=2, space="PSUM") as psB:
                for t in range(TT):
                    c, tl = divmod(t, TC)
                    psk = psB.tile([P, JQ], F32)
                    psv = psB.tile([P, JQ], F32)
                    for ei in range(ET):
                        lhsT = xsb[ei][c][:, ts(tl, P)]
                        nc.tensor.matmul(
                            psk[:], lhsT, wkvsb[ei][:, 0:JQ],
                            start=(ei == 0), stop=(ei == ET - 1),
                        )
                        nc.tensor.matmul(
                            psv[:], lhsT, wkvsb[ei][:, JQ:2 * JQ],
                            start=(ei == 0), stop=(ei == ET - 1),
                        )
                    nc.any.tensor_copy(out=kvsb[t][:, 0:JQ], in_=psk[:])
                    nc.any.tensor_copy(out=kvsb[t][:, JQ:2 * JQ], in_=psv[:])
                    # software-pipeline M-acc one t-tile behind the copies
                    if t >= 1:
                        macc(t - 1, psMt)
                macc(TT - 1, psMt)

            # scale + extract diag blocks into the zeroed bf16 send tile
            msb = msp.tile([P, NPAIR * P], BF16, name="msb")
            nc.vector.memset(msb[:], 0.0)
            for p in range(NPAIR):
                nc.vector.tensor_scalar_mul(
                    msb[0:D, p * P:p * P + D], psMt[p][0:D, 0:D], SCALE)
                nc.vector.tensor_scalar_mul(
                    msb[D:P, p * P + D:(p + 1) * P], psMt[p][D:P, D:P], SCALE)

        # ---------------- pair AllGather of M blocks ----------------
        bounce_in = dram.tile([P, NPAIR * P], BF16, name="cc_in")
        bounce_out = dram.tile([2 * P, NPAIR * P], BF16, name="cc_out")
        nc.gpsimd.dma_start(bounce_in[:], msb[:])
        nc.gpsimd.collective_compute(
            "AllGather",
            mybir.AluOpType.bypass,
            replica_groups=PAIR_GROUPS,
            ins=[bounce_in.opt()],
            outs=[bounce_out.opt()],
        )
        # all 8 global pairs, rank-major: rows 0:128 = rank0 pairs 0..3
        mall = msp.tile([P, GPAIR * P], BF16, name="mall")
        nc.sync.dma_start(mall[:, 0:NPAIR * P], bounce_out[0:P, :])
        nc.sync.dma_start(mall[:, NPAIR * P:GPAIR * P], bounce_out[P:2 * P, :])

        # ---------------- phase 2: AT then W ----------------
        atsb = [atp.tile([P, E], BF16, name="atsb") for _ in range(GPAIR)]
        wsb = [wsp.tile([P, OC], BF16, name="wsb") for _ in range(ET)]
        with (
            tc_.tile_pool(name="psA", bufs=4, space="PSUM") as psA,
            tc_.tile_pool(name="psC", bufs=3, space="PSUM") as psC,
        ):
            for g in range(GPAIR):
                for h in range(2):
                    pa = psA.tile([P, E // 2], F32)
                    nc.tensor.matmul(
                        pa[:], mall[:, ts(g, P)], wqsb[g][:, ts(h, E // 2)],
                        start=True, stop=True,
                    )
                    nc.any.tensor_copy(
                        out=atsb[g][:, ts(h, E // 2)], in_=pa[:])
            for ei in range(ET):
                pw = psC.tile([P, OC], F32)
                for g in range(GPAIR):
                    nc.tensor.matmul(
                        pw[:], atsb[g][:, ts(ei, P)], wosb[g][:],
                        start=(g == 0), stop=(g == GPAIR - 1),
                    )
                nc.any.tensor_copy(out=wsb[ei][:], in_=pw[:])

        # ---------------- phase 3: out = x @ W ----------------
        with tc_.tile_pool(name="psO", bufs=3, space="PSUM") as psO:
            for t in range(TT):
                c, tl = divmod(t, TC)
                po = psO.tile([P, OC], F32)
                for ei in range(ET):
                    nc.tensor.matmul(
                        po[:], xsb[ei][c][:, ts(tl, P)], wsb[ei][:],
                        start=(ei == 0), stop=(ei == ET - 1),
                    )
                ot = otp.tile([P, OC], F32, name="ot")
                nc.any.tensor_copy(out=ot[:], in_=po[:])
                nc.sync.dma_start(out[ts(t, P), :], ot[:])


def _build():
    nc = bacc.Bacc("TRN2", target_bir_lowering=False, debug=False,
                   num_devices=NCORES)
    xt = nc.dram_tensor("xt", [E, S], BF16, kind="ExternalInput").ap()
    wkv = nc.dram_tensor("wkv", [E, 2 * JQ], BF16, kind="ExternalInput").ap()
    wq = nc.dram_tensor("wq", [E, E], BF16, kind="ExternalInput").ap()
    wo = nc.dram_tensor("wo", [E, OC], BF16, kind="ExternalInput").ap()
    out = nc.dram_tensor("out", [S, OC], F32, kind="ExternalOutput").ap()

    with tile.TileContext(nc) as tc_:
        _emit(tc_, nc, xt, wkv, wq, wo, out)
    nc.compile()
    return nc


def _in_maps(x, w_qkv, w_o):
    import ml_dtypes
    bf = ml_dtypes.bfloat16
    xTs = [np.ascontiguousarray(x[b].T).astype(bf) for b in range(B)]
    # q rows in global head order: full wq for the AT step (same all cores)
    qrows_all = np.concatenate(
        [np.arange(192 * h, 192 * h + D) for h in range(H)])
    wq_ = np.ascontiguousarray(w_qkv[qrows_all]).astype(bf)   # [E, E]
    whalf = []
    for r in range(2):
        hs = range(r * HPC, (r + 1) * HPC)
        krows = np.concatenate(
            [np.arange(192 * h + D, 192 * h + 2 * D) for h in hs])
        vrows = krows + D
        wkv_ = np.ascontiguousarray(
            w_qkv[np.concatenate([krows, vrows])].T).astype(bf)  # [E, 1024]
        wo_ = np.ascontiguousarray(
            w_o[r * OC:(r + 1) * OC, :].T).astype(bf)            # [E, 512]
        whalf.append((wkv_, wo_))
    maps = []
    for core in range(NCORES):
        b, r = divmod(core, 2)
        wkv_, wo_ = whalf[r]
        maps.append({"xt": xTs[b], "wkv": wkv_, "wq": wq_, "wo": wo_})
    return maps


def _gather(results):
    full = np.empty((B, S, E), np.float32)
    for b in range(B):
        full[b, :, 0:OC] = results[2 * b]["out"]
        full[b, :, OC:E] = results[2 * b + 1]["out"]
    return full


def _run(x, w_qkv, w_o, trace=False):
    global _MODULE
    x = np.ascontiguousarray(np.asarray(x, dtype=np.float32))
    w_qkv = np.ascontiguousarray(np.asarray(w_qkv, dtype=np.float32))
    w_o = np.ascontiguousarray(np.asarray(w_o, dtype=np.float32))
    if _MODULE is None:
        _MODULE = _build()
    res = run_bass_kernel_spmd(
        _MODULE, _in_maps(x, w_qkv, w_o),
        core_ids=list(range(NCORES)), trace=trace,
    )
    return _gather(res.results), res


def kernel(x, w_qkv, w_o):
    out, _ = _run(x, w_qkv, w_o, trace=False)
    return out


# revision 9
# speedup vs baseline: 1.0552x; 1.0552x over previous
"""Trainium2 Bass kernel for nn_MultiHeadAttention_70549132804637.

Reference computation (note: NO softmax — values use raw scaled logits):
    qkv = x @ w_qkv.T                         (B,S,3E) split per head into q,k,v
    logits = (q @ k^T) * scale                (B,H,S,S)
    values = logits @ v                       (B,H,S,D)
    out    = concat_heads(values) @ w_o.T     (B,S,E)

Because there is no softmax the map is linear in x on the left:
    out = x @ W,  W = sum_h Wq_h^T M_h Wo_h^T,  M_h = scale k_h^T v_h
and M_h itself needs no per-token k/v:
    M_h = scale Wk_h^T (x^T x) Wv_h
so one Gram matrix G = x^T x (shared by all 16 heads) replaces the whole
qkv projection.  Per-core FLOPs drop from 8.9G (baseline) to 8.0G and
there is no cross-core traffic at all.

Sharding over 8 cores: core c handles batch b = c//2 and output-column
half r = c%2.  Every core computes G and all 16 heads' M_h, then builds
W[:, r*512:(r+1)*512] and out[:, r*512:(r+1)*512] = x_b @ W_half.  The
host concatenates column halves (no partial-sum adds).

Per-core phases (all-tokens T=2048, E=1024, D=64):
  G   : upper-triangular blocks of G = x^T x   (x natural layout,
        t-streamed as x arrives), lower blocks via PE transposes
  Y   : Y = G @ Wv_all                  (G-blocks stationary)
  M   : MT_g += Y-slices^T @ Wk-slices  (per head-pair, diag 64x64 blocks)
  Z   : Z_g = MT_g^T-blockdiag @ WoT_g  (tiny stationary M blocks)
  W   : W[e, oc] = wq-slices^T @ Z_g, acc over g
  out : out[t, oc] = xT-slices^T @ W[e], acc over e

All matmul operands bf16 (FWL fast weight loads); PSUM accumulates f32.
A short stream of dummy matmuls during the DMA lead-in keeps the PE's
HAM activity monitor busy so real matmuls start at the full clock.
"""

from contextlib import ExitStack

import numpy as np

import concourse.mybir as mybir
import concourse.tile as tile
from concourse import bacc
from concourse.bass import ts
from concourse.bass_utils import run_bass_kernel_spmd

F32 = mybir.dt.float32
BF16 = mybir.dt.bfloat16

B, S, E, H = 4, 2048, 1024, 16
D = E // H                 # 64 per-head dim
SCALE = 0.125              # 1/sqrt(D), exact power of two
NCORES = 8
P = 128
ET = E // P                # 8 e-tiles
TT = S // P                # 16 token tiles
GPAIR = H // 2             # 8 head pairs
OC = E // 2                # 512 output columns per core

_MODULE = None


def _emit(tc_, nc, xn, xt, wkv, wq, wo, eye, out):
    with ExitStack() as ctx:
        xnp = ctx.enter_context(tc_.tile_pool(name="xnp", bufs=TT))
        xtp = ctx.enter_context(tc_.tile_pool(name="xtp", bufs=ET))
        wkvp = ctx.enter_context(tc_.tile_pool(name="wkvp", bufs=ET))
        wqp = ctx.enter_context(tc_.tile_pool(name="wqp", bufs=GPAIR))
        wop = ctx.enter_context(tc_.tile_pool(name="wop", bufs=GPAIR))
        eyep = ctx.enter_context(tc_.tile_pool(name="eyep", bufs=1))
        gup = ctx.enter_context(tc_.tile_pool(name="gup", bufs=1))
        glop = ctx.enter_context(tc_.tile_pool(name="glop", bufs=1))
        yp = ctx.enter_context(tc_.tile_pool(name="yp", bufs=ET))
        mp = ctx.enter_context(tc_.tile_pool(name="mp", bufs=GPAIR))
        zp = ctx.enter_context(tc_.tile_pool(name="zp", bufs=GPAIR))
        wsp = ctx.enter_context(tc_.tile_pool(name="wsp", bufs=ET))
        otp = ctx.enter_context(tc_.tile_pool(name="otp", bufs=4))

        # ---------------- DMA in ----------------
        eyesb = eyep.tile([P, P], BF16, name="eyesb")
        nc.sync.dma_start(eyesb[:], eye[:])
        xnsb = [xnp.tile([P, E], BF16, name="xnsb") for _ in range(TT)]
        for t in range(TT):
            nc.sync.dma_start(xnsb[t][:], xn[ts(t, P), :])
        xtsb = [xtp.tile([P, S], BF16, name="xtsb") for _ in range(ET)]
        for ei in range(ET):
            nc.sync.dma_start(xtsb[ei][:], xt[ts(ei, P), :])
        # weights on the gpsimd queue so descriptor generation for x tiles
        # (the critical path) is not delayed
        wkvsb = [wkvp.tile([P, 2 * E], BF16, name="wkvsb") for _ in range(ET)]
        for ei in range(ET):
            nc.gpsimd.dma_start(wkvsb[ei][:], wkv[ts(ei, P), :])
        wqsb = [wqp.tile([P, E], BF16, name="wqsb") for _ in range(GPAIR)]
        wosb = [wop.tile([P, OC], BF16, name="wosb") for _ in range(GPAIR)]
        for g in range(GPAIR):
            nc.gpsimd.dma_start(wosb[g][:], wo[ts(g, P), :])
        for g in range(GPAIR):
            nc.gpsimd.dma_start(wqsb[g][:], wq[ts(g, P), :])

        # PE warm-up: dummy matmuls during the DMA head keep the HAM
        # activity monitor busy so real matmuls start at full clock
        with tc_.tile_pool(name="psWm", bufs=1, space="PSUM") as psWm:
            wps = psWm.tile([P, P], F32, name="wps")
            for _ in range(30):
                nc.tensor.matmul(wps[:], eyesb[:], eyesb[:],
                                 start=True, stop=True)

        # ---------------- phase G: upper blocks of x^T x ----------------
        # row i covers blocks (i, j) for j >= i, width E - 128*i
        gusb = [gup.tile([P, E - P * i], BF16, name=f"gu{i}")
                for i in range(ET)]

        def gpass(rows, pool):
            pstiles = {i: pool.tile([P, E - P * i], F32, name=f"psg{i}")
                       for i in rows}
            for t in range(TT):
                for i in rows:
                    w_ = E - P * i
                    lhsT = xnsb[t][:, ts(i, P)]
                    for off in range(0, w_, 512):
                        fw = min(512, w_ - off)
                        nc.tensor.matmul(
                            pstiles[i][:, off:off + fw],
                            lhsT,
                            xnsb[t][:, P * i + off:P * i + off + fw],
                            start=(t == 0), stop=(t == TT - 1),
                            skip_group_check=True,
                        )
            for i in rows:
                nc.any.tensor_copy(out=gusb[i][:], in_=pstiles[i][:])

        with tc_.tile_pool(name="psGA", bufs=1, space="PSUM") as psGA:
            gpass((0, 1, 2), psGA)
        with tc_.tile_pool(name="psGB", bufs=1, space="PSUM") as psGB:
            gpass((3, 4, 5, 6, 7), psGB)

        # lower blocks (b, a), b > a: PE transpose of upper (a, b)
        glsb = {}
        with tc_.tile_pool(name="psT", bufs=4, space="PSUM") as psT:
            for a in range(ET):
                for b_ in range(a + 1, ET):
                    pt = psT.tile([P, P], BF16)
                    nc.tensor.transpose(
                        pt[:], gusb[a][:, (b_ - a) * P:(b_ - a + 1) * P],
                        eyesb[:])
                    gl = glop.tile([P, P], BF16, name=f"gl{b_}_{a}")
                    nc.any.tensor_copy(out=gl[:], in_=pt[:])
                    glsb[(b_, a)] = gl

        # ---------------- phase Y: Y = G @ Wv ----------------
        ysb = [yp.tile([P, E], BF16, name="ysb") for _ in range(ET)]
        with tc_.tile_pool(name="psY", bufs=2, space="PSUM") as psY:
            for a in range(ET):
                py = psY.tile([P, E], F32)
                for b_ in range(ET):
                    lhsT = (gusb[b_][:, (a - b_) * P:(a - b_ + 1) * P]
                            if b_ <= a else glsb[(b_, a)][:])
                    for h in range(2):
                        nc.tensor.matmul(
                            py[:, ts(h, 512)], lhsT,
                            wkvsb[b_][:, E + 512 * h:E + 512 * (h + 1)],
                            start=(b_ == 0), stop=(b_ == ET - 1),
                            skip_group_check=True,
                        )
                nc.any.tensor_copy(out=ysb[a][:], in_=py[:])

        # ---------------- phase M: MT_g = Y_g^T @ Wk_g ----------------
        mblk = [mp.tile([P, P], BF16, name="mblk") for _ in range(GPAIR)]
        with tc_.tile_pool(name="psM", bufs=GPAIR, space="PSUM") as psM:
            psm = [psM.tile([P, P], F32, name="psm") for _ in range(GPAIR)]
            for g in range(GPAIR):
                for e in range(ET):
                    nc.tensor.matmul(
                        psm[g][:], ysb[e][:, ts(g, P)],
                        wkvsb[e][:, ts(g, P)],
                        start=(e == 0), stop=(e == ET - 1),
                    )
            for g in range(GPAIR):
                nc.vector.memset(mblk[g][:], 0.0)
                nc.vector.tensor_scalar_mul(
                    mblk[g][0:D, 0:D], psm[g][0:D, 0:D], SCALE)
                nc.vector.tensor_scalar_mul(
                    mblk[g][D:P, D:P], psm[g][D:P, D:P], SCALE)

        # ---------------- phase Z + W ----------------
        zsb = [zp.tile([P, OC], BF16, name="zsb") for _ in range(GPAIR)]
        wsb = [wsp.tile([P, OC], BF16, name="wsb") for _ in range(ET)]
        with (
            tc_.tile_pool(name="psZ", bufs=2, space="PSUM") as psZ,
            tc_.tile_pool(name="psC", bufs=3, space="PSUM") as psC,
        ):
            for g in range(GPAIR):
                pz = psZ.tile([P, OC], F32)
                nc.tensor.matmul(pz[:], mblk[g][:], wosb[g][:],
                                 start=True, stop=True)
                nc.any.tensor_copy(out=zsb[g][:], in_=pz[:])
            for ei in range(ET):
                pw = psC.tile([P, OC], F32)
                for g in range(GPAIR):
                    nc.tensor.matmul(
                        pw[:], wqsb[g][:, ts(ei, P)], zsb[g][:],
                        start=(g == 0), stop=(g == GPAIR - 1),
                    )
                nc.any.tensor_copy(out=wsb[ei][:], in_=pw[:])

        # ---------------- phase out: out = x @ W ----------------
        with tc_.tile_pool(name="psO", bufs=3, space="PSUM") as psO:
            for t in range(TT):
                po = psO.tile([P, OC], F32)
                for ei in range(ET):
                    nc.tensor.matmul(
                        po[:], xtsb[ei][:, ts(t, P)], wsb[ei][:],
                        start=(ei == 0), stop=(ei == ET - 1),
                    )
                ot = otp.tile([P, OC], F32, name="ot")
                nc.any.tensor_copy(out=ot[:], in_=po[:])
                nc.sync.dma_start(out[ts(t, P), :], ot[:])


def _build():
    nc = bacc.Bacc("TRN2", target_bir_lowering=False, debug=False,
                   num_devices=NCORES)
    xn = nc.dram_tensor("xn", [S, E], BF16, kind="ExternalInput").ap()
    xt = nc.dram_tensor("xt", [E, S], BF16, kind="ExternalInput").ap()
    wkv = nc.dram_tensor("wkv", [E, 2 * E], BF16, kind="ExternalInput").ap()
    wq = nc.dram_tensor("wq", [E, E], BF16, kind="ExternalInput").ap()
    wo = nc.dram_tensor("wo", [E, OC], BF16, kind="ExternalInput").ap()
    eye = nc.dram_tensor("eye", [P, P], BF16, kind="ExternalInput").ap()
    out = nc.dram_tensor("out", [S, OC], F32, kind="ExternalOutput").ap()

    with tile.TileContext(nc) as tc_:
        _emit(tc_, nc, xn, xt, wkv, wq, wo, eye, out)
    nc.compile()
    return nc


def _in_maps(x, w_qkv, w_o):
    import ml_dtypes
    bf = ml_dtypes.bfloat16
    # global head order row indices
    qrows = np.concatenate([np.arange(192 * h, 192 * h + D) for h in range(H)])
    krows = qrows + D
    vrows = qrows + 2 * D
    wq_ = np.ascontiguousarray(w_qkv[qrows]).astype(bf)          # [E, E]
    wkv_ = np.ascontiguousarray(
        np.concatenate([w_qkv[krows].T, w_qkv[vrows].T], axis=1)
    ).astype(bf)                                                  # [E, 2E]
    eye_ = np.eye(P, dtype=bf)
    xns = [np.ascontiguousarray(x[b]).astype(bf) for b in range(B)]
    xts = [np.ascontiguousarray(x[b].T).astype(bf) for b in range(B)]
    wos = [np.ascontiguousarray(w_o[r * OC:(r + 1) * OC, :].T).astype(bf)
           for r in range(2)]                                     # [E, 512]
    maps = []
    for core in range(NCORES):
        b, r = divmod(core, 2)
        maps.append({"xn": xns[b], "xt": xts[b], "wkv": wkv_,
                     "wq": wq_, "wo": wos[r], "eye": eye_})
    return maps


def _gather(results):
    full = np.empty((B, S, E), np.float32)
    for b in range(B):
        full[b, :, 0:OC] = results[2 * b]["out"]
        full[b, :, OC:E] = results[2 * b + 1]["out"]
    return full


def _run(x, w_qkv, w_o, trace=False):
    global _MODULE
    x = np.ascontiguousarray(np.asarray(x, dtype=np.float32))
    w_qkv = np.ascontiguousarray(np.asarray(w_qkv, dtype=np.float32))
    w_o = np.ascontiguousarray(np.asarray(w_o, dtype=np.float32))
    if _MODULE is None:
        _MODULE = _build()
    res = run_bass_kernel_spmd(
        _MODULE, _in_maps(x, w_qkv, w_o),
        core_ids=list(range(NCORES)), trace=trace,
    )
    return _gather(res.results), res


def kernel(x, w_qkv, w_o):
    out, _ = _run(x, w_qkv, w_o, trace=False)
    return out


# revision 16
# speedup vs baseline: 1.0993x; 1.0418x over previous
"""Trainium2 Bass kernel for nn_MultiHeadAttention_70549132804637.

Reference computation (note: NO softmax — values use raw scaled logits):
    qkv = x @ w_qkv.T                         (B,S,3E) split per head into q,k,v
    logits = (q @ k^T) * scale                (B,H,S,S)
    values = logits @ v                       (B,H,S,D)
    out    = concat_heads(values) @ w_o.T     (B,S,E)

Because there is no softmax the map is linear in x on the left:
    out = x @ W,  W = sum_h Wq_h^T M_h Wo_h^T,  M_h = scale k_h^T v_h
and M_h itself needs no per-token k/v:
    M_h = scale Wk_h^T (x^T x) Wv_h
so one Gram matrix G = x^T x (shared by all 16 heads) replaces the whole
qkv projection.  Per-core FLOPs drop from 8.9G (baseline) to 8.0G and
there is no cross-core traffic at all.

Sharding over 8 cores: core c handles batch b = c//2 and output-column
half r = c%2.  Every core computes G and all 16 heads' M_h, then builds
W[:, r*512:(r+1)*512] and out[:, r*512:(r+1)*512] = x_b @ W_half.  The
host concatenates column halves (no partial-sum adds).

Per-core phases (all-tokens T=2048, E=1024, D=64):
  G   : upper-triangular blocks of G = x^T x   (x natural layout,
        t-streamed as x arrives), lower blocks via PE transposes
  Y   : Y = G @ Wv_all                  (G-blocks stationary)
  M   : MT_g += Y-slices^T @ Wk-slices  (per head-pair, diag 64x64 blocks)
  Z   : Z_g = MT_g^T-blockdiag @ WoT_g  (tiny stationary M blocks)
  W   : W[e, oc] = wq-slices^T @ Z_g, acc over g
  out : out[t, oc] = xT-slices^T @ W[e], acc over e

All matmul operands bf16 (FWL fast weight loads); PSUM accumulates f32.
A short stream of dummy matmuls during the DMA lead-in keeps the PE's
HAM activity monitor busy so real matmuls start at the full clock.
"""

from contextlib import ExitStack

import numpy as np

import concourse.mybir as mybir
import concourse.tile as tile
from concourse import bacc
from concourse.bass import ts
from concourse.bass_utils import run_bass_kernel_spmd

F32 = mybir.dt.float32
BF16 = mybir.dt.bfloat16

B, S, E, H = 4, 2048, 1024, 16
D = E // H                 # 64 per-head dim
SCALE = 0.125              # 1/sqrt(D), exact power of two
NCORES = 8
P = 128
ET = E // P                # 8 e-tiles
TT = S // P                # 16 token tiles
GPAIR = H // 2             # 8 head pairs
OC = E // 2                # 512 output columns per core

_MODULE = None


def _emit(tc_, nc, xn, xt, wkv, wq, wo, eye, out):
    with ExitStack() as ctx:
        xnp = ctx.enter_context(tc_.tile_pool(name="xnp", bufs=TT))
        xtp = ctx.enter_context(tc_.tile_pool(name="xtp", bufs=ET))
        wkvp = ctx.enter_context(tc_.tile_pool(name="wkvp", bufs=ET))
        wqp = ctx.enter_context(tc_.tile_pool(name="wqp", bufs=GPAIR))
        wop = ctx.enter_context(tc_.tile_pool(name="wop", bufs=GPAIR))
        eyep = ctx.enter_context(tc_.tile_pool(name="eyep", bufs=1))
        gup = ctx.enter_context(tc_.tile_pool(name="gup", bufs=1))
        glop = ctx.enter_context(tc_.tile_pool(name="glop", bufs=1))
        yp = ctx.enter_context(tc_.tile_pool(name="yp", bufs=ET))
        mp = ctx.enter_context(tc_.tile_pool(name="mp", bufs=GPAIR))
        zp = ctx.enter_context(tc_.tile_pool(name="zp", bufs=GPAIR))
        wsp = ctx.enter_context(tc_.tile_pool(name="wsp", bufs=ET))
        otp = ctx.enter_context(tc_.tile_pool(name="otp", bufs=6))


        def copy_alt(n, out_, in_):
            if n % 2 == 0:
                nc.scalar.copy(out_, in_)
            else:
                nc.vector.tensor_copy(out=out_, in_=in_)

        # ---------------- DMA in ----------------
        xnsb = [xnp.tile([P, E], BF16, name="xnsb") for _ in range(TT)]
        for t in range(TT):
            nc.sync.dma_start(xnsb[t][:], xn[ts(t, P), :])
        xtsb = [xtp.tile([P, S], BF16, name="xtsb") for _ in range(ET)]
        for ei in range(ET):
            nc.sync.dma_start(xtsb[ei][:], xt[ts(ei, P), :])
        eyesb = eyep.tile([P, P], BF16, name="eyesb")
        nc.sync.dma_start(eyesb[:], eye[:])
        # weights on the gpsimd queue so descriptor generation for x tiles
        # (the critical path) is not delayed
        wkvsb = [wkvp.tile([P, 2 * E], BF16, name="wkvsb") for _ in range(ET)]
        for ei in range(ET):
            nc.gpsimd.dma_start(wkvsb[ei][:], wkv[ts(ei, P), :])
        wqsb = [wqp.tile([P, E], BF16, name="wqsb") for _ in range(GPAIR)]
        wosb = [wop.tile([P, OC], BF16, name="wosb") for _ in range(GPAIR)]
        for g in range(GPAIR):
            nc.gpsimd.dma_start(wosb[g][:], wo[ts(g, P), :])
        for g in range(GPAIR):
            nc.gpsimd.dma_start(wqsb[g][:], wq[ts(g, P), :])

        # PE warm-up on a memset tile (no DMA dependency): dummy matmuls
        # during the DMA head keep the HAM activity monitor busy so real
        # matmuls start at full clock
        warm = ctx.enter_context(tc_.tile_pool(name="warm", bufs=1))
        wmt = warm.tile([P, P], BF16, name="wmt")
        nc.gpsimd.memset(wmt[:], 0.0)
        with tc_.tile_pool(name="psWm", bufs=1, space="PSUM") as psWm:
            wps = psWm.tile([P, P], F32, name="wps")
            for _ in range(40):
                nc.tensor.matmul(wps[:], wmt[:], wmt[:],
                                 start=True, stop=True)

        # ---------------- phase G: upper blocks of x^T x ----------------
        # row i covers blocks (i, j) for j >= i, width E - 128*i
        gusb = [gup.tile([P, E - P * i], BF16, name=f"gu{i}")
                for i in range(ET)]

        def gpass(rows, pool):
            pstiles = {i: pool.tile([P, E - P * i], F32, name=f"psg{i}")
                       for i in rows}
            for t in range(TT):
                for i in rows:
                    w_ = E - P * i
                    lhsT = xnsb[t][:, ts(i, P)]
                    for off in range(0, w_, 512):
                        fw = min(512, w_ - off)
                        nc.tensor.matmul(
                            pstiles[i][:, off:off + fw],
                            lhsT,
                            xnsb[t][:, P * i + off:P * i + off + fw],
                            start=(t == 0), stop=(t == TT - 1),
                            skip_group_check=True,
                        )
            for n_, i in enumerate(rows):
                copy_alt(n_, gusb[i][:], pstiles[i][:])

        with tc_.tile_pool(name="psGA", bufs=1, space="PSUM") as psGA:
            gpass((0, 1, 2), psGA)
        with tc_.tile_pool(name="psGB", bufs=1, space="PSUM") as psGB:
            gpass((3, 4, 5, 6, 7), psGB)

        # lower blocks (b, a), b > a: PE transpose of upper (a, b)
        glsb = {}
        with tc_.tile_pool(name="psT", bufs=6, space="PSUM") as psT:
            n_ = 0
            for a in range(ET):
                for b_ in range(a + 1, ET):
                    pt = psT.tile([P, P], BF16)
                    nc.tensor.transpose(
                        pt[:], gusb[a][:, (b_ - a) * P:(b_ - a + 1) * P],
                        eyesb[:])
                    gl = glop.tile([P, P], BF16, name=f"gl{b_}_{a}")
                    copy_alt(n_, gl[:], pt[:])
                    glsb[(b_, a)] = gl
                    n_ += 1

        # ---------------- phase Y: Y = G @ Wv ----------------
        ysb = [yp.tile([P, E], BF16, name="ysb") for _ in range(ET)]
        with tc_.tile_pool(name="psY", bufs=3, space="PSUM") as psY:
            for a in range(ET):
                py = psY.tile([P, E], F32)
                for b_ in range(ET):
                    lhsT = (gusb[b_][:, (a - b_) * P:(a - b_ + 1) * P]
                            if b_ <= a else glsb[(b_, a)][:])
                    for h in range(2):
                        nc.tensor.matmul(
                            py[:, ts(h, 512)], lhsT,
                            wkvsb[b_][:, E + 512 * h:E + 512 * (h + 1)],
                            start=(b_ == 0), stop=(b_ == ET - 1),
                            skip_group_check=True,
                        )
                copy_alt(a, ysb[a][:], py[:])

        # ---------------- phase M: MT_g = Y_g^T @ Wk_g ----------------
        # e-outer so accumulation starts as soon as each ysb tile lands
        mblk = [mp.tile([P, P], BF16, name="mblk") for _ in range(GPAIR)]
        with tc_.tile_pool(name="psM", bufs=GPAIR, space="PSUM") as psM:
            psm = [psM.tile([P, P], F32, name="psm") for _ in range(GPAIR)]
            for e in range(ET):
                for g in range(GPAIR):
                    nc.tensor.matmul(
                        psm[g][:], ysb[e][:, ts(g, P)],
                        wkvsb[e][:, ts(g, P)],
                        start=(e == 0), stop=(e == ET - 1),
                        skip_group_check=True,
                    )
            for g in range(GPAIR):
                nc.vector.memset(mblk[g][:], 0.0)
                nc.vector.tensor_scalar_mul(
                    mblk[g][0:D, 0:D], psm[g][0:D, 0:D], SCALE)
                nc.vector.tensor_scalar_mul(
                    mblk[g][D:P, D:P], psm[g][D:P, D:P], SCALE)

        # ---------------- phase Z + W ----------------
        zsb = [zp.tile([P, OC], BF16, name="zsb") for _ in range(GPAIR)]
        wsb = [wsp.tile([P, OC], BF16, name="wsb") for _ in range(ET)]
        with (
            tc_.tile_pool(name="psZ", bufs=3, space="PSUM") as psZ,
            tc_.tile_pool(name="psC", bufs=4, space="PSUM") as psC,
        ):
            for g in range(GPAIR):
                pz = psZ.tile([P, OC], F32)
                nc.tensor.matmul(pz[:], mblk[g][:], wosb[g][:],
                                 start=True, stop=True)
                copy_alt(g, zsb[g][:], pz[:])
            for ei in range(ET):
                pw = psC.tile([P, OC], F32)
                for g in range(GPAIR):
                    nc.tensor.matmul(
                        pw[:], wqsb[g][:, ts(ei, P)], zsb[g][:],
                        start=(g == 0), stop=(g == GPAIR - 1),
                    )
                copy_alt(ei, wsb[ei][:], pw[:])

        # ---------------- phase out: out = x @ W ----------------
        with tc_.tile_pool(name="psO", bufs=6, space="PSUM") as psO:
            for t in range(TT):
                po = psO.tile([P, OC], F32)
                for ei in range(ET):
                    nc.tensor.matmul(
                        po[:], xtsb[ei][:, ts(t, P)], wsb[ei][:],
                        start=(ei == 0), stop=(ei == ET - 1),
                    )
                ot = otp.tile([P, OC], BF16, name="ot")
                copy_alt(t, ot[:], po[:])
                nc.sync.dma_start(out[ts(t, P), :], ot[:])


def _build():
    nc = bacc.Bacc("TRN2", target_bir_lowering=False, debug=False,
                   num_devices=NCORES)
    xn = nc.dram_tensor("xn", [S, E], BF16, kind="ExternalInput").ap()
    xt = nc.dram_tensor("xt", [E, S], BF16, kind="ExternalInput").ap()
    wkv = nc.dram_tensor("wkv", [E, 2 * E], BF16, kind="ExternalInput").ap()
    wq = nc.dram_tensor("wq", [E, E], BF16, kind="ExternalInput").ap()
    wo = nc.dram_tensor("wo", [E, OC], BF16, kind="ExternalInput").ap()
    eye = nc.dram_tensor("eye", [P, P], BF16, kind="ExternalInput").ap()
    out = nc.dram_tensor("out", [S, OC], BF16, kind="ExternalOutput").ap()

    with tile.TileContext(nc) as tc_:
        _emit(tc_, nc, xn, xt, wkv, wq, wo, eye, out)
    nc.compile()
    return nc


def _in_maps(x, w_qkv, w_o):
    import ml_dtypes
    bf = ml_dtypes.bfloat16
    # global head order row indices
    qrows = np.concatenate([np.arange(192 * h, 192 * h + D) for h in range(H)])
    krows = qrows + D
    vrows = qrows + 2 * D
    wq_ = np.ascontiguousarray(w_qkv[qrows]).astype(bf)          # [E, E]
    wkv_ = np.ascontiguousarray(
        np.concatenate([w_qkv[krows].T, w_qkv[vrows].T], axis=1)
    ).astype(bf)                                                  # [E, 2E]
    eye_ = np.eye(P, dtype=bf)
    xns = [np.ascontiguousarray(x[b]).astype(bf) for b in range(B)]
    xts = [np.ascontiguousarray(x[b].T).astype(bf) for b in range(B)]
    wos = [np.ascontiguousarray(w_o[r * OC:(r + 1) * OC, :].T).astype(bf)
           for r in range(2)]                                     # [E, 512]
    maps = []
    for core in range(NCORES):
        b, r = divmod(core, 2)
        maps.append({"xn": xns[b], "xt": xts[b], "wkv": wkv_,
                     "wq": wq_, "wo": wos[r], "eye": eye_})
    return maps


def _gather(results):
    full = np.empty((B, S, E), np.float32)
    for b in range(B):
        full[b, :, 0:OC] = results[2 * b]["out"].astype(np.float32)
        full[b, :, OC:E] = results[2 * b + 1]["out"].astype(np.float32)
    return full


def _run(x, w_qkv, w_o, trace=False):
    global _MODULE
    x = np.ascontiguousarray(np.asarray(x, dtype=np.float32))
    w_qkv = np.ascontiguousarray(np.asarray(w_qkv, dtype=np.float32))
    w_o = np.ascontiguousarray(np.asarray(w_o, dtype=np.float32))
    if _MODULE is None:
        _MODULE = _build()
    res = run_bass_kernel_spmd(
        _MODULE, _in_maps(x, w_qkv, w_o),
        core_ids=list(range(NCORES)), trace=trace,
    )
    return _gather(res.results), res


def kernel(x, w_qkv, w_o):
    out, _ = _run(x, w_qkv, w_o, trace=False)
    return out


# revision 18
# speedup vs baseline: 1.1994x; 1.0911x over previous
"""Trainium2 Bass kernel for nn_MultiHeadAttention_70549132804637.

Reference computation (note: NO softmax — values use raw scaled logits):
    qkv = x @ w_qkv.T                         (B,S,3E) split per head into q,k,v
    logits = (q @ k^T) * scale                (B,H,S,S)
    values = logits @ v                       (B,H,S,D)
    out    = concat_heads(values) @ w_o.T     (B,S,E)

Because there is no softmax the map is linear in x on the left:
    out = x @ W,  W = sum_h Wq_h^T M_h Wo_h^T,  M_h = scale k_h^T v_h
and M_h itself needs no per-token k/v:
    M_h = scale Wk_h^T (x^T x) Wv_h
so one Gram matrix G = x^T x (shared by all 16 heads) replaces the whole
qkv projection.  Per-core FLOPs drop from 8.9G (baseline) to 8.0G and
there is no cross-core traffic at all.

Sharding over 8 cores: core c handles batch b = c//2 and output-column
half r = c%2.  Every core computes G and all 16 heads' M_h, then builds
W[:, r*512:(r+1)*512] and out[:, r*512:(r+1)*512] = x_b @ W_half.  The
host concatenates column halves (no partial-sum adds).

Per-core phases (all-tokens T=2048, E=1024, D=64):
  G   : upper-triangular blocks of G = x^T x   (x natural layout,
        t-streamed as x arrives), lower blocks via PE transposes
  Y   : Y = G @ Wv_all                  (G-blocks stationary)
  M   : MT_g += Y-slices^T @ Wk-slices  (per head-pair, diag 64x64 blocks)
  Z   : Z_g = MT_g^T-blockdiag @ WoT_g  (tiny stationary M blocks)
  W   : W[e, oc] = wq-slices^T @ Z_g, acc over g
  out : out[t, oc] = xT-slices^T @ W[e], acc over e

All matmul operands bf16 (FWL fast weight loads); PSUM accumulates f32.
A short stream of dummy matmuls during the DMA lead-in keeps the PE's
HAM activity monitor busy so real matmuls start at the full clock.
"""

from contextlib import ExitStack

import numpy as np

import concourse.mybir as mybir
import concourse.tile as tile
from concourse import bacc
from concourse.bass import ts
from concourse.bass_utils import run_bass_kernel_spmd

F32 = mybir.dt.float32
BF16 = mybir.dt.bfloat16

B, S, E, H = 4, 2048, 1024, 16
D = E // H                 # 64 per-head dim
SCALE = 0.125              # 1/sqrt(D), exact power of two
NCORES = 8
P = 128
ET = E // P                # 8 e-tiles
TT = S // P                # 16 token tiles
GPAIR = H // 2             # 8 head pairs
OC = E // 2                # 512 output columns per core

_MODULE = None


def _emit(tc_, nc, xn, xt, wkv, wq, wo, eye, out):
    with ExitStack() as ctx:
        xnp = ctx.enter_context(tc_.tile_pool(name="xnp", bufs=TT))
        xtp = ctx.enter_context(tc_.tile_pool(name="xtp", bufs=ET))
        wkvp = ctx.enter_context(tc_.tile_pool(name="wkvp", bufs=ET))
        wqp = ctx.enter_context(tc_.tile_pool(name="wqp", bufs=GPAIR))
        wop = ctx.enter_context(tc_.tile_pool(name="wop", bufs=GPAIR))
        eyep = ctx.enter_context(tc_.tile_pool(name="eyep", bufs=1))
        gup = ctx.enter_context(tc_.tile_pool(name="gup", bufs=1))
        glop = ctx.enter_context(tc_.tile_pool(name="glop", bufs=1))
        yp = ctx.enter_context(tc_.tile_pool(name="yp", bufs=ET))
        mp = ctx.enter_context(tc_.tile_pool(name="mp", bufs=GPAIR))
        zp = ctx.enter_context(tc_.tile_pool(name="zp", bufs=GPAIR))
        wsp = ctx.enter_context(tc_.tile_pool(name="wsp", bufs=ET))
        otp = ctx.enter_context(tc_.tile_pool(name="otp", bufs=6))


        def copy_alt(n, out_, in_):
            if n % 2 == 0:
                nc.scalar.copy(out_, in_)
            else:
                nc.vector.tensor_copy(out=out_, in_=in_)

        # ---------------- DMA in ----------------
        # memset the warm-up tile FIRST on gpsimd, before that queue is
        # loaded with weight-DMA descriptor generation, so the PE warm-up
        # can start immediately
        warm = ctx.enter_context(tc_.tile_pool(name="warm", bufs=1))
        wmt = warm.tile([P, P], BF16, name="wmt")
        nc.gpsimd.memset(wmt[:], 0.0)
        xnsb = [xnp.tile([P, E], BF16, name="xnsb") for _ in range(TT)]
        for t in range(TT):
            nc.sync.dma_start(xnsb[t][:], xn[ts(t, P), :])
        xtsb = [xtp.tile([P, S], BF16, name="xtsb") for _ in range(ET)]
        for ei in range(ET):
            nc.sync.dma_start(xtsb[ei][:], xt[ts(ei, P), :])
        eyesb = eyep.tile([P, P], BF16, name="eyesb")
        nc.sync.dma_start(eyesb[:], eye[:])
        # weights on the gpsimd queue so descriptor generation for x tiles
        # (the critical path) is not delayed
        wkvsb = [wkvp.tile([P, 2 * E], BF16, name="wkvsb") for _ in range(ET)]
        for ei in range(ET):
            nc.gpsimd.dma_start(wkvsb[ei][:], wkv[ts(ei, P), :])
        wqsb = [wqp.tile([P, E], BF16, name="wqsb") for _ in range(GPAIR)]
        wosb = [wop.tile([P, OC], BF16, name="wosb") for _ in range(GPAIR)]
        for g in range(GPAIR):
            nc.gpsimd.dma_start(wosb[g][:], wo[ts(g, P), :])
        for g in range(GPAIR):
            nc.gpsimd.dma_start(wqsb[g][:], wq[ts(g, P), :])

        # PE warm-up on the memset tile (no DMA dependency): dummy matmuls
        # during the DMA head keep the HAM activity monitor busy so real
        # matmuls start at full clock
        with tc_.tile_pool(name="psWm", bufs=1, space="PSUM") as psWm:
            wps = psWm.tile([P, P], F32, name="wps")
            for _ in range(40):
                nc.tensor.matmul(wps[:], wmt[:], wmt[:],
                                 start=True, stop=True)

        # ---------------- phase G: upper blocks of x^T x ----------------
        # row i covers blocks (i, j) for j >= i, width E - 128*i
        gusb = [gup.tile([P, E - P * i], BF16, name=f"gu{i}")
                for i in range(ET)]

        def gpass(rows, pool):
            pstiles = {i: pool.tile([P, E - P * i], F32, name=f"psg{i}")
                       for i in rows}
            for t in range(TT):
                for i in rows:
                    w_ = E - P * i
                    lhsT = xnsb[t][:, ts(i, P)]
                    for off in range(0, w_, 512):
                        fw = min(512, w_ - off)
                        nc.tensor.matmul(
                            pstiles[i][:, off:off + fw],
                            lhsT,
                            xnsb[t][:, P * i + off:P * i + off + fw],
                            start=(t == 0), stop=(t == TT - 1),
                            skip_group_check=True,
                        )
            for n_, i in enumerate(rows):
                copy_alt(n_, gusb[i][:], pstiles[i][:])

        with tc_.tile_pool(name="psGA", bufs=1, space="PSUM") as psGA:
            gpass((0, 1, 2), psGA)
        with tc_.tile_pool(name="psGB", bufs=1, space="PSUM") as psGB:
            gpass((3, 4, 5, 6, 7), psGB)

        # lower blocks (b, a), b > a: PE transpose of upper (a, b)
        glsb = {}
        with tc_.tile_pool(name="psT", bufs=6, space="PSUM") as psT:
            n_ = 0
            for a in range(ET):
                for b_ in range(a + 1, ET):
                    pt = psT.tile([P, P], BF16)
                    nc.tensor.transpose(
                        pt[:], gusb[a][:, (b_ - a) * P:(b_ - a + 1) * P],
                        eyesb[:])
                    gl = glop.tile([P, P], BF16, name=f"gl{b_}_{a}")
                    copy_alt(n_, gl[:], pt[:])
                    glsb[(b_, a)] = gl
                    n_ += 1

        # ---------------- phase Y: Y = G @ Wv ----------------
        ysb = [yp.tile([P, E], BF16, name="ysb") for _ in range(ET)]
        with tc_.tile_pool(name="psY", bufs=3, space="PSUM") as psY:
            for a in range(ET):
                py = psY.tile([P, E], F32)
                for b_ in range(ET):
                    lhsT = (gusb[b_][:, (a - b_) * P:(a - b_ + 1) * P]
                            if b_ <= a else glsb[(b_, a)][:])
                    for h in range(2):
                        nc.tensor.matmul(
                            py[:, ts(h, 512)], lhsT,
                            wkvsb[b_][:, E + 512 * h:E + 512 * (h + 1)],
                            start=(b_ == 0), stop=(b_ == ET - 1),
                            skip_group_check=True,
                        )
                copy_alt(a, ysb[a][:], py[:])

        # ---------------- phase M: MT_g = Y_g^T @ Wk_g ----------------
        # e-outer so accumulation starts as soon as each ysb tile lands
        mblk = [mp.tile([P, P], BF16, name="mblk") for _ in range(GPAIR)]
        with tc_.tile_pool(name="psM", bufs=GPAIR, space="PSUM") as psM:
            psm = [psM.tile([P, P], F32, name="psm") for _ in range(GPAIR)]
            for e in range(ET):
                for g in range(GPAIR):
                    nc.tensor.matmul(
                        psm[g][:], ysb[e][:, ts(g, P)],
                        wkvsb[e][:, ts(g, P)],
                        start=(e == 0), stop=(e == ET - 1),
                        skip_group_check=True,
                    )
            for g in range(GPAIR):
                nc.vector.memset(mblk[g][:], 0.0)
                nc.vector.tensor_scalar_mul(
                    mblk[g][0:D, 0:D], psm[g][0:D, 0:D], SCALE)
                nc.vector.tensor_scalar_mul(
                    mblk[g][D:P, D:P], psm[g][D:P, D:P], SCALE)

        # ---------------- phase Z + W ----------------
        zsb = [zp.tile([P, OC], BF16, name="zsb") for _ in range(GPAIR)]
        wsb = [wsp.tile([P, OC], BF16, name="wsb") for _ in range(ET)]
        with (
            tc_.tile_pool(name="psZ", bufs=3, space="PSUM") as psZ,
            tc_.tile_pool(name="psC", bufs=4, space="PSUM") as psC,
        ):
            for g in range(GPAIR):
                pz = psZ.tile([P, OC], F32)
                nc.tensor.matmul(pz[:], mblk[g][:], wosb[g][:],
                                 start=True, stop=True)
                copy_alt(g, zsb[g][:], pz[:])
            for ei in range(ET):
                pw = psC.tile([P, OC], F32)
                for g in range(GPAIR):
                    nc.tensor.matmul(
                        pw[:], wqsb[g][:, ts(ei, P)], zsb[g][:],
                        start=(g == 0), stop=(g == GPAIR - 1),
                    )
                copy_alt(ei, wsb[ei][:], pw[:])

        # ---------------- phase out: out = x @ W ----------------
        with tc_.tile_pool(name="psO", bufs=6, space="PSUM") as psO:
            for t in range(TT):
                po = psO.tile([P, OC], F32)
                for ei in range(ET):
                    nc.tensor.matmul(
                        po[:], xtsb[ei][:, ts(t, P)], wsb[ei][:],
                        start=(ei == 0), stop=(ei == ET - 1),
                    )
                ot = otp.tile([P, OC], BF16, name="ot")
                copy_alt(t, ot[:], po[:])
                nc.sync.dma_start(out[ts(t, P), :], ot[:])


def _build():
    nc = bacc.Bacc("TRN2", target_bir_lowering=False, debug=False,
                   num_devices=NCORES)
    xn = nc.dram_tensor("xn", [S, E], BF16, kind="ExternalInput").ap()
    xt = nc.dram_tensor("xt", [E, S], BF16, kind="ExternalInput").ap()
    wkv = nc.dram_tensor("wkv", [E, 2 * E], BF16, kind="ExternalInput").ap()
    wq = nc.dram_tensor("wq", [E, E], BF16, kind="ExternalInput").ap()
    wo = nc.dram_tensor("wo", [E, OC], BF16, kind="ExternalInput").ap()
    eye = nc.dram_tensor("eye", [P, P], BF16, kind="ExternalInput").ap()
    out = nc.dram_tensor("out", [S, OC], BF16, kind="ExternalOutput").ap()

    with tile.TileContext(nc) as tc_:
        _emit(tc_, nc, xn, xt, wkv, wq, wo, eye, out)
    nc.compile()
    return nc


def _in_maps(x, w_qkv, w_o):
    import ml_dtypes
    bf = ml_dtypes.bfloat16
    # global head order row indices
    qrows = np.concatenate([np.arange(192 * h, 192 * h + D) for h in range(H)])
    krows = qrows + D
    vrows = qrows + 2 * D
    wq_ = np.ascontiguousarray(w_qkv[qrows]).astype(bf)          # [E, E]
    wkv_ = np.ascontiguousarray(
        np.concatenate([w_qkv[krows].T, w_qkv[vrows].T], axis=1)
    ).astype(bf)                                                  # [E, 2E]
    eye_ = np.eye(P, dtype=bf)
    xns = [np.ascontiguousarray(x[b]).astype(bf) for b in range(B)]
    xts = [np.ascontiguousarray(x[b].T).astype(bf) for b in range(B)]
    wos = [np.ascontiguousarray(w_o[r * OC:(r + 1) * OC, :].T).astype(bf)
           for r in range(2)]                                     # [E, 512]
    maps = []
    for core in range(NCORES):
        b, r = divmod(core, 2)
        maps.append({"xn": xns[b], "xt": xts[b], "wkv": wkv_,
                     "wq": wq_, "wo": wos[r], "eye": eye_})
    return maps


def _gather(results):
    full = np.empty((B, S, E), np.float32)
    for b in range(B):
        full[b, :, 0:OC] = results[2 * b]["out"].astype(np.float32)
        full[b, :, OC:E] = results[2 * b + 1]["out"].astype(np.float32)
    return full


def _run(x, w_qkv, w_o, trace=False):
    global _MODULE
    x = np.ascontiguousarray(np.asarray(x, dtype=np.float32))
    w_qkv = np.ascontiguousarray(np.asarray(w_qkv, dtype=np.float32))
    w_o = np.ascontiguousarray(np.asarray(w_o, dtype=np.float32))
    if _MODULE is None:
        _MODULE = _build()
    res = run_bass_kernel_spmd(
        _MODULE, _in_maps(x, w_qkv, w_o),
        core_ids=list(range(NCORES)), trace=trace,
    )
    return _gather(res.results), res


def kernel(x, w_qkv, w_o):
    out, _ = _run(x, w_qkv, w_o, trace=False)
    return out


# revision 20
# speedup vs baseline: 1.2588x; 1.0495x over previous
"""Trainium2 Bass kernel for nn_MultiHeadAttention_70549132804637.

Reference computation (note: NO softmax — values use raw scaled logits):
    qkv = x @ w_qkv.T                         (B,S,3E) split per head into q,k,v
    logits = (q @ k^T) * scale                (B,H,S,S)
    values = logits @ v                       (B,H,S,D)
    out    = concat_heads(values) @ w_o.T     (B,S,E)

Because there is no softmax the map is linear in x on the left:
    out = x @ W,  W = sum_h Wq_h^T M_h Wo_h^T,  M_h = scale k_h^T v_h
and M_h itself needs no per-token k/v:
    M_h = scale Wk_h^T (x^T x) Wv_h
so one Gram matrix G = x^T x (shared by all 16 heads) replaces the whole
qkv projection.  Per-core FLOPs drop from 8.9G (baseline) to 8.0G and
there is no cross-core traffic at all.

Sharding over 8 cores: core c handles batch b = c//2 and output-column
half r = c%2.  Every core computes G and all 16 heads' M_h, then builds
W[:, r*512:(r+1)*512] and out[:, r*512:(r+1)*512] = x_b @ W_half.  The
host concatenates column halves (no partial-sum adds).

Per-core phases (all-tokens T=2048, E=1024, D=64):
  G   : upper-triangular blocks of G = x^T x   (x natural layout,
        t-streamed as x arrives), lower blocks via PE transposes
  Y   : Y = G @ Wv_all                  (G-blocks stationary)
  M   : MT_g += Y-slices^T @ Wk-slices  (per head-pair, diag 64x64 blocks)
  Z   : Z_g = MT_g^T-blockdiag @ WoT_g  (tiny stationary M blocks)
  W   : W[e, oc] = wq-slices^T @ Z_g, acc over g
  out : out[t, oc] = xT-slices^T @ W[e], acc over e

Performance notes (measured on HW):
  - matmul pitch is free-dim cycles at ~2.37 GHz with no per-instruction
    overhead as long as the tensor queue never blocks on semaphores;
  - DMA engines are packet-rate bound (~270ns per packet per engine,
    16 engines), so every host tensor is retiled into [128, 4096] bf16
    chunks => 8 KB contiguous per partition per packet (~full bandwidth);
  - PSUM->SBUF copies are split in half across the scalar and vector
    engines so no single queue becomes the drain bottleneck;
  - PE warm-up matmuls run on a memset tile (memset is the FIRST gpsimd
    instruction, ahead of that queue's DMA descriptor work) to keep the
    HAM activity monitor busy during the DMA lead-in.
"""

from contextlib import ExitStack

import numpy as np

import concourse.mybir as mybir
import concourse.tile as tile
from concourse import bacc
from concourse.bass import ts
from concourse.bass_utils import run_bass_kernel_spmd

F32 = mybir.dt.float32
BF16 = mybir.dt.bfloat16

B, S, E, H = 4, 2048, 1024, 16
D = E // H                 # 64 per-head dim
SCALE = 0.125              # 1/sqrt(D), exact power of two
NCORES = 8
P = 128
ET = E // P                # 8 e-tiles
TT = S // P                # 16 token tiles
GPAIR = H // 2             # 8 head pairs
OC = E // 2                # 512 output columns per core
CW = 4096                  # big-chunk width (8 KB/partition in bf16)

_MODULE = None


def _emit(tc_, nc, xn, xt, wkv, wq, wo, eye, out):
    with ExitStack() as ctx:
        xnp = ctx.enter_context(tc_.tile_pool(name="xnp", bufs=4))
        xtp = ctx.enter_context(tc_.tile_pool(name="xtp", bufs=4))
        wkvp = ctx.enter_context(tc_.tile_pool(name="wkvp", bufs=4))
        wqp = ctx.enter_context(tc_.tile_pool(name="wqp", bufs=2))
        wop = ctx.enter_context(tc_.tile_pool(name="wop", bufs=1))
        eyep = ctx.enter_context(tc_.tile_pool(name="eyep", bufs=1))
        warm = ctx.enter_context(tc_.tile_pool(name="warm", bufs=1))
        gup = ctx.enter_context(tc_.tile_pool(name="gup", bufs=1))
        glop = ctx.enter_context(tc_.tile_pool(name="glop", bufs=1))
        yp = ctx.enter_context(tc_.tile_pool(name="yp", bufs=ET))
        mp = ctx.enter_context(tc_.tile_pool(name="mp", bufs=GPAIR))
        zp = ctx.enter_context(tc_.tile_pool(name="zp", bufs=GPAIR))
        wsp = ctx.enter_context(tc_.tile_pool(name="wsp", bufs=ET))
        otp = ctx.enter_context(tc_.tile_pool(name="otp", bufs=2))

        def copy2(out_ap, in_ap, w):
            # split a [128, w] copy across scalar+vector so neither queue
            # becomes the drain bottleneck
            h = w // 2
            nc.scalar.copy(out_ap[:, 0:h], in_ap[:, 0:h])
            nc.vector.tensor_copy(out=out_ap[:, h:w], in_=in_ap[:, h:w])

        # ---------------- DMA in ----------------
        # memset the warm-up tile FIRST on gpsimd, before that queue is
        # loaded with weight-DMA descriptor generation
        wmt = warm.tile([P, P], BF16, name="wmt")
        nc.gpsimd.memset(wmt[:], 0.0)
        # all host tensors are retiled into [128, 4096] chunks
        xng = [xnp.tile([P, CW], BF16, name="xng") for _ in range(4)]
        for c in range(4):
            nc.sync.dma_start(xng[c][:], xn[ts(c, P), :])
        eyesb = eyep.tile([P, P], BF16, name="eyesb")
        nc.sync.dma_start(eyesb[:], eye[:])
        xtg = [xtp.tile([P, CW], BF16, name="xtg") for _ in range(4)]
        for c in range(4):
            nc.sync.dma_start(xtg[c][:], xt[ts(c, P), :])
        wkvg = [wkvp.tile([P, CW], BF16, name="wkvg") for _ in range(4)]
        for c in range(4):
            nc.gpsimd.dma_start(wkvg[c][:], wkv[ts(c, P), :])
        wog = wop.tile([P, CW], BF16, name="wog")
        nc.gpsimd.dma_start(wog[:], wo[0:P, :])
        wqg = [wqp.tile([P, CW], BF16, name="wqg") for _ in range(2)]
        for c in range(2):
            nc.gpsimd.dma_start(wqg[c][:], wq[ts(c, P), :])

        # chunk-layout accessors (w = slice width in elements)
        def xn_sl(t, a, b):
            return xng[t // 4][:, (t % 4) * E + a:(t % 4) * E + b]

        def xt_sl(ei, a, b):
            return xtg[ei // 2][:, (ei % 2) * S + a:(ei % 2) * S + b]

        def wkv_sl(ei, a, b):
            return wkvg[ei // 2][:, (ei % 2) * 2 * E + a:(ei % 2) * 2 * E + b]

        def wq_sl(g, a, b):
            return wqg[g // 4][:, (g % 4) * E + a:(g % 4) * E + b]

        def wo_sl(g):
            return wog[:, g * OC:(g + 1) * OC]

        # Mblk staging zeroed early (vector queue is idle here)
        mblk = [mp.tile([P, P], BF16, name="mblk") for _ in range(GPAIR)]
        for g in range(GPAIR):
            nc.vector.memset(mblk[g][:], 0.0)

        # PE warm-up: dummy matmuls during the DMA head keep the HAM
        # activity monitor busy so real matmuls start at full clock
        with tc_.tile_pool(name="psWm", bufs=1, space="PSUM") as psWm:
            wps = psWm.tile([P, P], F32, name="wps")
            for _ in range(40):
                nc.tensor.matmul(wps[:], wmt[:], wmt[:],
                                 start=True, stop=True)

        # ---------------- phase G: upper blocks of x^T x ----------------
        # row i covers blocks (i, j) for j >= i, width E - 128*i
        gusb = [gup.tile([P, E - P * i], BF16, name=f"gu{i}")
                for i in range(ET)]
        glsb = {}

        def gpass(rows, pool):
            pstiles = {i: pool.tile([P, E - P * i], F32, name=f"psg{i}")
                       for i in rows}
            for t in range(TT):
                for i in rows:
                    w_ = E - P * i
                    lhsT = xn_sl(t, i * P, (i + 1) * P)
                    for off in range(0, w_, 512):
                        fw = min(512, w_ - off)
                        nc.tensor.matmul(
                            pstiles[i][:, off:off + fw],
                            lhsT,
                            xn_sl(t, P * i + off, P * i + off + fw),
                            start=(t == 0), stop=(t == TT - 1),
                            skip_group_check=True,
                        )
            for i in rows:
                copy2(gusb[i], pstiles[i], E - P * i)

        def tpass(rows, pool):
            # lower blocks (b, a), b > a: PE transpose of upper (a, b)
            for a in rows:
                for b_ in range(a + 1, ET):
                    pt = pool.tile([P, P], BF16)
                    nc.tensor.transpose(
                        pt[:], gusb[a][:, (b_ - a) * P:(b_ - a + 1) * P],
                        eyesb[:])
                    gl = glop.tile([P, P], BF16, name=f"gl{b_}_{a}")
                    if (a + b_) % 2 == 0:
                        nc.scalar.copy(gl[:], pt[:])
                    else:
                        nc.vector.tensor_copy(out=gl[:], in_=pt[:])
                    glsb[(b_, a)] = gl

        with tc_.tile_pool(name="psGA", bufs=1, space="PSUM") as psGA:
            gpass((0, 1, 2), psGA)
        with tc_.tile_pool(name="psTA", bufs=2, space="PSUM") as psTA:
            with tc_.tile_pool(name="psGB", bufs=1, space="PSUM") as psGB:
                tpass((0, 1, 2), psTA)
                gpass((3, 4, 5, 6, 7), psGB)
        with tc_.tile_pool(name="psTB", bufs=4, space="PSUM") as psTB:
            tpass((3, 4, 5, 6, 7), psTB)

        # ---------------- phase Y: Y = G @ Wv ----------------
        ysb = [yp.tile([P, E], BF16, name="ysb") for _ in range(ET)]
        with tc_.tile_pool(name="psY", bufs=3, space="PSUM") as psY:
            for a in range(ET):
                py = psY.tile([P, E], F32)
                for b_ in range(ET):
                    lhsT = (gusb[b_][:, (a - b_) * P:(a - b_ + 1) * P]
                            if b_ <= a else glsb[(b_, a)][:])
                    for h in range(2):
                        nc.tensor.matmul(
                            py[:, ts(h, 512)], lhsT,
                            wkv_sl(b_, E + 512 * h, E + 512 * (h + 1)),
                            start=(b_ == 0), stop=(b_ == ET - 1),
                            skip_group_check=True,
                        )
                copy2(ysb[a], py, E)

        # ---------------- phase M: MT_g = Y_g^T @ Wk_g ----------------
        # e-outer so accumulation starts as soon as each ysb tile lands
        with tc_.tile_pool(name="psM", bufs=GPAIR, space="PSUM") as psM:
            psm = [psM.tile([P, P], F32, name="psm") for _ in range(GPAIR)]
            for e in range(ET):
                for g in range(GPAIR):
                    nc.tensor.matmul(
                        psm[g][:], ysb[e][:, ts(g, P)],
                        wkv_sl(e, g * P, (g + 1) * P),
                        start=(e == 0), stop=(e == ET - 1),
                        skip_group_check=True,
                    )
            # scale + extract the diag 64x64 blocks (MT per head)
            for g in range(GPAIR):
                nc.vector.tensor_scalar_mul(
                    mblk[g][0:D, 0:D], psm[g][0:D, 0:D], SCALE)
                nc.vector.tensor_scalar_mul(
                    mblk[g][D:P, D:P], psm[g][D:P, D:P], SCALE)

        # ---------------- phase Z + W ----------------
        zsb = [zp.tile([P, OC], BF16, name="zsb") for _ in range(GPAIR)]
        wsb = [wsp.tile([P, OC], BF16, name="wsb") for _ in range(ET)]
        with (
            tc_.tile_pool(name="psZ", bufs=3, space="PSUM") as psZ,
            tc_.tile_pool(name="psC", bufs=4, space="PSUM") as psC,
        ):
            for g in range(GPAIR):
                pz = psZ.tile([P, OC], F32)
                nc.tensor.matmul(pz[:], mblk[g][:], wo_sl(g),
                                 start=True, stop=True)
                copy2(zsb[g], pz, OC)
            for ei in range(ET):
                pw = psC.tile([P, OC], F32)
                for g in range(GPAIR):
                    nc.tensor.matmul(
                        pw[:], wq_sl(g, ei * P, (ei + 1) * P), zsb[g][:],
                        start=(g == 0), stop=(g == GPAIR - 1),
                    )
                copy2(wsb[ei], pw, OC)

        # ---------------- phase out: out = x @ W ----------------
        # batch 4 t-tiles into one [128, 2048] staging tile per DMA
        with tc_.tile_pool(name="psO", bufs=6, space="PSUM") as psO:
            for c in range(4):
                og = otp.tile([P, 4 * OC], BF16, name="og")
                for tl in range(4):
                    t = 4 * c + tl
                    po = psO.tile([P, OC], F32)
                    for ei in range(ET):
                        nc.tensor.matmul(
                            po[:], xt_sl(ei, t * P, (t + 1) * P), wsb[ei][:],
                            start=(ei == 0), stop=(ei == ET - 1),
                        )
                    copy2(og[:, tl * OC:(tl + 1) * OC], po, OC)
                nc.sync.dma_start(out[ts(c, P), :], og[:])


def _build():
    nc = bacc.Bacc("TRN2", target_bir_lowering=False, debug=False,
                   num_devices=NCORES)
    # all inputs retiled by the host into [n*128, 4096] chunk layouts
    xn = nc.dram_tensor("xn", [4 * P, CW], BF16, kind="ExternalInput").ap()
    xt = nc.dram_tensor("xt", [4 * P, CW], BF16, kind="ExternalInput").ap()
    wkv = nc.dram_tensor("wkv", [4 * P, CW], BF16, kind="ExternalInput").ap()
    wq = nc.dram_tensor("wq", [2 * P, CW], BF16, kind="ExternalInput").ap()
    wo = nc.dram_tensor("wo", [P, CW], BF16, kind="ExternalInput").ap()
    eye = nc.dram_tensor("eye", [P, P], BF16, kind="ExternalInput").ap()
    out = nc.dram_tensor("out", [4 * P, 4 * OC], BF16,
                         kind="ExternalOutput").ap()

    with tile.TileContext(nc) as tc_:
        _emit(tc_, nc, xn, xt, wkv, wq, wo, eye, out)
    nc.compile()
    return nc


def _pack(a, k):
    # [n*128, w] -> chunks of k row-tiles side by side: [n/k*128, k*w]
    n, w = a.shape
    assert n % (k * P) == 0 and k * w == CW
    return np.ascontiguousarray(
        a.reshape(n // (k * P), k, P, w).transpose(0, 2, 1, 3)
        .reshape(n // k, k * w))


def _in_maps(x, w_qkv, w_o):
    import ml_dtypes
    bf = ml_dtypes.bfloat16
    # global head order row indices
    qrows = np.concatenate([np.arange(192 * h, 192 * h + D) for h in range(H)])
    krows = qrows + D
    vrows = qrows + 2 * D
    wq_ = _pack(w_qkv[qrows], 4).astype(bf)                      # [256, 4096]
    wkv_ = _pack(np.concatenate(
        [w_qkv[krows].T, w_qkv[vrows].T], axis=1), 2).astype(bf)  # [512, 4096]
    eye_ = np.eye(P, dtype=bf)
    xns = [_pack(x[b], 4).astype(bf) for b in range(B)]           # [512, 4096]
    xts = [_pack(np.ascontiguousarray(x[b].T), 2).astype(bf)
           for b in range(B)]                                     # [512, 4096]
    wos = [_pack(np.ascontiguousarray(
        w_o[r * OC:(r + 1) * OC, :].T), 8).astype(bf)
        for r in range(2)]                                        # [128, 4096]
    maps = []
    for core in range(NCORES):
        b, r = divmod(core, 2)
        maps.append({"xn": xns[b], "xt": xts[b], "wkv": wkv_,
                     "wq": wq_, "wo": wos[r], "eye": eye_})
    return maps


def _gather(results):
    full = np.empty((B, S, E), np.float32)
    for b in range(B):
        for r in range(2):
            og = results[2 * b + r]["out"].astype(np.float32)  # [512, 2048]
            full[b, :, r * OC:(r + 1) * OC] = (
                og.reshape(4, P, 4, OC).transpose(0, 2, 1, 3)
                .reshape(S, OC))
    return full


def _run(x, w_qkv, w_o, trace=False):
    global _MODULE
    x = np.ascontiguousarray(np.asarray(x, dtype=np.float32))
    w_qkv = np.ascontiguousarray(np.asarray(w_qkv, dtype=np.float32))
    w_o = np.ascontiguousarray(np.asarray(w_o, dtype=np.float32))
    if _MODULE is None:
        _MODULE = _build()
    res = run_bass_kernel_spmd(
        _MODULE, _in_maps(x, w_qkv, w_o),
        core_ids=list(range(NCORES)), trace=trace,
    )
    return _gather(res.results), res


def kernel(x, w_qkv, w_o):
    out, _ = _run(x, w_qkv, w_o, trace=False)
    return out


# revision 22
# speedup vs baseline: 1.2931x; 1.0272x over previous
"""Trainium2 Bass kernel for nn_MultiHeadAttention_70549132804637.

Reference computation (note: NO softmax — values use raw scaled logits):
    qkv = x @ w_qkv.T                         (B,S,3E) split per head into q,k,v
    logits = (q @ k^T) * scale                (B,H,S,S)
    values = logits @ v                       (B,H,S,D)
    out    = concat_heads(values) @ w_o.T     (B,S,E)

Because there is no softmax the map is linear in x on the left:
    out = x @ W,  W = sum_h Wq_h^T M_h Wo_h^T,  M_h = scale k_h^T v_h
and M_h itself needs no per-token k/v:
    M_h = scale Wk_h^T (x^T x) Wv_h
so one Gram matrix G = x^T x (shared by all 16 heads) replaces the whole
qkv projection.  Per-core FLOPs drop from 8.9G (baseline) to 8.0G and
there is no cross-core traffic at all.

Sharding over 8 cores: core c handles batch b = c//2 and output-column
half r = c%2.  Every core computes G and all 16 heads' M_h, then builds
W[:, r*512:(r+1)*512] and out[:, r*512:(r+1)*512] = x_b @ W_half.  The
host concatenates column halves (no partial-sum adds).

Per-core phases (all-tokens T=2048, E=1024, D=64):
  G   : upper-triangular blocks of G = x^T x   (x natural layout,
        t-streamed as x arrives), lower blocks via PE transposes
  Y   : Y = G @ Wv_all                  (G-blocks stationary)
  M   : MT_g += Y-slices^T @ Wk-slices  (per head-pair, diag 64x64 blocks)
  Z   : Z_g = MT_g^T-blockdiag @ WoT_g  (tiny stationary M blocks)
  W   : W[e, oc] = wq-slices^T @ Z_g, acc over g
  out : out[t, oc] = xT-slices^T @ W[e], acc over e

Performance notes (measured on HW):
  - matmul pitch is free-dim cycles at ~2.37 GHz with no per-instruction
    overhead as long as the tensor queue never blocks on semaphores;
  - DMA engines are packet-rate bound (~270ns per packet per engine,
    16 engines), so every host tensor is retiled into [128, 4096] bf16
    chunks => 8 KB contiguous per partition per packet (~full bandwidth);
  - PSUM->SBUF copies are split in half across the scalar and vector
    engines so no single queue becomes the drain bottleneck;
  - PE warm-up matmuls run on a memset tile (memset is the FIRST gpsimd
    instruction, ahead of that queue's DMA descriptor work) to keep the
    HAM activity monitor busy during the DMA lead-in.
"""

from contextlib import ExitStack

import numpy as np

import concourse.mybir as mybir
import concourse.tile as tile
from concourse import bacc
from concourse.bass import ts
from concourse.bass_utils import run_bass_kernel_spmd

F32 = mybir.dt.float32
BF16 = mybir.dt.bfloat16

B, S, E, H = 4, 2048, 1024, 16
D = E // H                 # 64 per-head dim
SCALE = 0.125              # 1/sqrt(D), exact power of two
NCORES = 8
P = 128
ET = E // P                # 8 e-tiles
TT = S // P                # 16 token tiles
GPAIR = H // 2             # 8 head pairs
OC = E // 2                # 512 output columns per core
CW = 4096                  # big-chunk width (8 KB/partition in bf16)

_MODULE = None


def _emit(tc_, nc, xn, xt, wkv, wq, wo, eye, out):
    with ExitStack() as ctx:
        xnp = ctx.enter_context(tc_.tile_pool(name="xnp", bufs=4))
        xtp = ctx.enter_context(tc_.tile_pool(name="xtp", bufs=4))
        wkvp = ctx.enter_context(tc_.tile_pool(name="wkvp", bufs=4))
        wqp = ctx.enter_context(tc_.tile_pool(name="wqp", bufs=2))
        wop = ctx.enter_context(tc_.tile_pool(name="wop", bufs=1))
        eyep = ctx.enter_context(tc_.tile_pool(name="eyep", bufs=1))
        warm = ctx.enter_context(tc_.tile_pool(name="warm", bufs=1))
        gup = ctx.enter_context(tc_.tile_pool(name="gup", bufs=1))
        glop = ctx.enter_context(tc_.tile_pool(name="glop", bufs=1))
        yp = ctx.enter_context(tc_.tile_pool(name="yp", bufs=ET))
        mp = ctx.enter_context(tc_.tile_pool(name="mp", bufs=GPAIR))
        zp = ctx.enter_context(tc_.tile_pool(name="zp", bufs=GPAIR))
        wsp = ctx.enter_context(tc_.tile_pool(name="wsp", bufs=ET))
        otp = ctx.enter_context(tc_.tile_pool(name="otp", bufs=2))

        def copy2(out_ap, in_ap, w):
            # split a [128, w] copy across scalar+vector so neither queue
            # becomes the drain bottleneck
            h = w // 2
            nc.scalar.copy(out_ap[:, 0:h], in_ap[:, 0:h])
            nc.vector.tensor_copy(out=out_ap[:, h:w], in_=in_ap[:, h:w])

        # ---------------- DMA in ----------------
        # memset the warm-up tile FIRST on gpsimd, before that queue is
        # loaded with DMA descriptor generation
        wmt = warm.tile([P, P], BF16, name="wmt")
        nc.gpsimd.memset(wmt[:], 0.0)
        # Two hardware DGE queues exist (sync + scalar), each ~177 GB/s
        # with 8 KB packets; gpsimd drives a software queue.  Split the
        # critical-path xn chunks across both hw queues, wkv behind them
        # on scalar; late-needed tensors (xt, wq, wo) go on gpsimd.
        xng = [xnp.tile([P, CW], BF16, name="xng") for _ in range(4)]
        eyesb = eyep.tile([P, P], BF16, name="eyesb")
        nc.sync.dma_start(xng[0][:], xn[ts(0, P), :])
        nc.scalar.dma_start(xng[1][:], xn[ts(1, P), :])
        nc.sync.dma_start(xng[2][:], xn[ts(2, P), :])
        nc.scalar.dma_start(xng[3][:], xn[ts(3, P), :])
        nc.sync.dma_start(eyesb[:], eye[:])
        wkvg = [wkvp.tile([P, CW], BF16, name="wkvg") for _ in range(4)]
        for c in range(4):
            [nc.sync, nc.scalar][c % 2].dma_start(wkvg[c][:], wkv[ts(c, P), :])
        xtg = [xtp.tile([P, CW], BF16, name="xtg") for _ in range(4)]
        for c in range(4):
            nc.gpsimd.dma_start(xtg[c][:], xt[ts(c, P), :])
        wog = wop.tile([P, CW], BF16, name="wog")
        nc.gpsimd.dma_start(wog[:], wo[0:P, :])
        wqg = [wqp.tile([P, CW], BF16, name="wqg") for _ in range(2)]
        for c in range(2):
            nc.gpsimd.dma_start(wqg[c][:], wq[ts(c, P), :])

        # chunk-layout accessors (w = slice width in elements)
        def xn_sl(t, a, b):
            return xng[t // 4][:, (t % 4) * E + a:(t % 4) * E + b]

        def xt_sl(ei, a, b):
            return xtg[ei // 2][:, (ei % 2) * S + a:(ei % 2) * S + b]

        def wkv_sl(ei, a, b):
            return wkvg[ei // 2][:, (ei % 2) * 2 * E + a:(ei % 2) * 2 * E + b]

        def wq_sl(g, a, b):
            return wqg[g // 4][:, (g % 4) * E + a:(g % 4) * E + b]

        def wo_sl(g):
            return wog[:, g * OC:(g + 1) * OC]

        # Mblk staging zeroed early (vector queue is idle here)
        mblk = [mp.tile([P, P], BF16, name="mblk") for _ in range(GPAIR)]
        for g in range(GPAIR):
            nc.vector.memset(mblk[g][:], 0.0)

        # PE warm-up: dummy matmuls during the DMA head keep the HAM
        # activity monitor busy so real matmuls start at full clock
        with tc_.tile_pool(name="psWm", bufs=1, space="PSUM") as psWm:
            wps = psWm.tile([P, P], F32, name="wps")
            for _ in range(40):
                nc.tensor.matmul(wps[:], wmt[:], wmt[:],
                                 start=True, stop=True)

        # ---------------- phase G: upper blocks of x^T x ----------------
        # row i covers blocks (i, j) for j >= i, width E - 128*i
        gusb = [gup.tile([P, E - P * i], BF16, name=f"gu{i}")
                for i in range(ET)]
        glsb = {}

        def gpass(rows, pool):
            pstiles = {i: pool.tile([P, E - P * i], F32, name=f"psg{i}")
                       for i in rows}
            for t in range(TT):
                for i in rows:
                    w_ = E - P * i
                    lhsT = xn_sl(t, i * P, (i + 1) * P)
                    for off in range(0, w_, 512):
                        fw = min(512, w_ - off)
                        nc.tensor.matmul(
                            pstiles[i][:, off:off + fw],
                            lhsT,
                            xn_sl(t, P * i + off, P * i + off + fw),
                            start=(t == 0), stop=(t == TT - 1),
                            skip_group_check=True,
                        )
            for i in rows:
                copy2(gusb[i], pstiles[i], E - P * i)

        def tpass(rows, pool):
            # lower blocks (b, a), b > a: PE transpose of upper (a, b)
            for a in rows:
                for b_ in range(a + 1, ET):
                    pt = pool.tile([P, P], BF16)
                    nc.tensor.transpose(
                        pt[:], gusb[a][:, (b_ - a) * P:(b_ - a + 1) * P],
                        eyesb[:])
                    gl = glop.tile([P, P], BF16, name=f"gl{b_}_{a}")
                    if (a + b_) % 2 == 0:
                        nc.scalar.copy(gl[:], pt[:])
                    else:
                        nc.vector.tensor_copy(out=gl[:], in_=pt[:])
                    glsb[(b_, a)] = gl

        with tc_.tile_pool(name="psGA", bufs=1, space="PSUM") as psGA:
            gpass((0, 1, 2), psGA)
        with tc_.tile_pool(name="psTA", bufs=2, space="PSUM") as psTA:
            with tc_.tile_pool(name="psGB", bufs=1, space="PSUM") as psGB:
                tpass((0, 1, 2), psTA)
                gpass((3, 4, 5, 6, 7), psGB)
        with tc_.tile_pool(name="psTB", bufs=4, space="PSUM") as psTB:
            tpass((3, 4, 5, 6, 7), psTB)

        # ---------------- phase Y: Y = G @ Wv ----------------
        ysb = [yp.tile([P, E], BF16, name="ysb") for _ in range(ET)]
        with tc_.tile_pool(name="psY", bufs=3, space="PSUM") as psY:
            for a in range(ET):
                py = psY.tile([P, E], F32)
                for b_ in range(ET):
                    lhsT = (gusb[b_][:, (a - b_) * P:(a - b_ + 1) * P]
                            if b_ <= a else glsb[(b_, a)][:])
                    for h in range(2):
                        nc.tensor.matmul(
                            py[:, ts(h, 512)], lhsT,
                            wkv_sl(b_, E + 512 * h, E + 512 * (h + 1)),
                            start=(b_ == 0), stop=(b_ == ET - 1),
                            skip_group_check=True,
                        )
                copy2(ysb[a], py, E)

        # ---------------- phase M: MT_g = Y_g^T @ Wk_g ----------------
        # e-outer so accumulation starts as soon as each ysb tile lands
        with tc_.tile_pool(name="psM", bufs=GPAIR, space="PSUM") as psM:
            psm = [psM.tile([P, P], F32, name="psm") for _ in range(GPAIR)]
            for e in range(ET):
                for g in range(GPAIR):
                    nc.tensor.matmul(
                        psm[g][:], ysb[e][:, ts(g, P)],
                        wkv_sl(e, g * P, (g + 1) * P),
                        start=(e == 0), stop=(e == ET - 1),
                        skip_group_check=True,
                    )
            # scale + extract the diag 64x64 blocks (MT per head)
            for g in range(GPAIR):
                nc.vector.tensor_scalar_mul(
                    mblk[g][0:D, 0:D], psm[g][0:D, 0:D], SCALE)
                nc.vector.tensor_scalar_mul(
                    mblk[g][D:P, D:P], psm[g][D:P, D:P], SCALE)

        # ---------------- phase Z + W ----------------
        zsb = [zp.tile([P, OC], BF16, name="zsb") for _ in range(GPAIR)]
        wsb = [wsp.tile([P, OC], BF16, name="wsb") for _ in range(ET)]
        with (
            tc_.tile_pool(name="psZ", bufs=3, space="PSUM") as psZ,
            tc_.tile_pool(name="psC", bufs=4, space="PSUM") as psC,
        ):
            for g in range(GPAIR):
                pz = psZ.tile([P, OC], F32)
                nc.tensor.matmul(pz[:], mblk[g][:], wo_sl(g),
                                 start=True, stop=True)
                copy2(zsb[g], pz, OC)
            for ei in range(ET):
                pw = psC.tile([P, OC], F32)
                for g in range(GPAIR):
                    nc.tensor.matmul(
                        pw[:], wq_sl(g, ei * P, (ei + 1) * P), zsb[g][:],
                        start=(g == 0), stop=(g == GPAIR - 1),
                    )
                copy2(wsb[ei], pw, OC)

        # ---------------- phase out: out = x @ W ----------------
        # batch 4 t-tiles into one [128, 2048] staging tile per DMA
        with tc_.tile_pool(name="psO", bufs=6, space="PSUM") as psO:
            for c in range(4):
                og = otp.tile([P, 4 * OC], BF16, name="og")
                for tl in range(4):
                    t = 4 * c + tl
                    po = psO.tile([P, OC], F32)
                    for ei in range(ET):
                        nc.tensor.matmul(
                            po[:], xt_sl(ei, t * P, (t + 1) * P), wsb[ei][:],
                            start=(ei == 0), stop=(ei == ET - 1),
                        )
                    copy2(og[:, tl * OC:(tl + 1) * OC], po, OC)
                # alternate hw queues so the last two output DMAs overlap
                [nc.sync, nc.scalar][c % 2].dma_start(out[ts(c, P), :], og[:])


def _build():
    nc = bacc.Bacc("TRN2", target_bir_lowering=False, debug=False,
                   num_devices=NCORES)
    # all inputs retiled by the host into [n*128, 4096] chunk layouts
    xn = nc.dram_tensor("xn", [4 * P, CW], BF16, kind="ExternalInput").ap()
    xt = nc.dram_tensor("xt", [4 * P, CW], BF16, kind="ExternalInput").ap()
    wkv = nc.dram_tensor("wkv", [4 * P, CW], BF16, kind="ExternalInput").ap()
    wq = nc.dram_tensor("wq", [2 * P, CW], BF16, kind="ExternalInput").ap()
    wo = nc.dram_tensor("wo", [P, CW], BF16, kind="ExternalInput").ap()
    eye = nc.dram_tensor("eye", [P, P], BF16, kind="ExternalInput").ap()
    out = nc.dram_tensor("out", [4 * P, 4 * OC], BF16,
                         kind="ExternalOutput").ap()

    with tile.TileContext(nc) as tc_:
        _emit(tc_, nc, xn, xt, wkv, wq, wo, eye, out)
    nc.compile()
    return nc


def _pack(a, k):
    # [n*128, w] -> chunks of k row-tiles side by side: [n/k*128, k*w]
    n, w = a.shape
    assert n % (k * P) == 0 and k * w == CW
    return np.ascontiguousarray(
        a.reshape(n // (k * P), k, P, w).transpose(0, 2, 1, 3)
        .reshape(n // k, k * w))


def _in_maps(x, w_qkv, w_o):
    import ml_dtypes
    bf = ml_dtypes.bfloat16
    # global head order row indices
    qrows = np.concatenate([np.arange(192 * h, 192 * h + D) for h in range(H)])
    krows = qrows + D
    vrows = qrows + 2 * D
    wq_ = _pack(w_qkv[qrows], 4).astype(bf)                      # [256, 4096]
    wkv_ = _pack(np.concatenate(
        [w_qkv[krows].T, w_qkv[vrows].T], axis=1), 2).astype(bf)  # [512, 4096]
    eye_ = np.eye(P, dtype=bf)
    xns = [_pack(x[b], 4).astype(bf) for b in range(B)]           # [512, 4096]
    xts = [_pack(np.ascontiguousarray(x[b].T), 2).astype(bf)
           for b in range(B)]                                     # [512, 4096]
    wos = [_pack(np.ascontiguousarray(
        w_o[r * OC:(r + 1) * OC, :].T), 8).astype(bf)
        for r in range(2)]                                        # [128, 4096]
    maps = []
    for core in range(NCORES):
        b, r = divmod(core, 2)
        maps.append({"xn": xns[b], "xt": xts[b], "wkv": wkv_,
                     "wq": wq_, "wo": wos[r], "eye": eye_})
    return maps


def _gather(results):
    full = np.empty((B, S, E), np.float32)
    for b in range(B):
        for r in range(2):
            og = results[2 * b + r]["out"].astype(np.float32)  # [512, 2048]
            full[b, :, r * OC:(r + 1) * OC] = (
                og.reshape(4, P, 4, OC).transpose(0, 2, 1, 3)
                .reshape(S, OC))
    return full


def _run(x, w_qkv, w_o, trace=False):
    global _MODULE
    x = np.ascontiguousarray(np.asarray(x, dtype=np.float32))
    w_qkv = np.ascontiguousarray(np.asarray(w_qkv, dtype=np.float32))
    w_o = np.ascontiguousarray(np.asarray(w_o, dtype=np.float32))
    if _MODULE is None:
        _MODULE = _build()
    res = run_bass_kernel_spmd(
        _MODULE, _in_maps(x, w_qkv, w_o),
        core_ids=list(range(NCORES)), trace=trace,
    )
    return _gather(res.results), res


def kernel(x, w_qkv, w_o):
    out, _ = _run(x, w_qkv, w_o, trace=False)
    return out
